# revision 16
# baseline (speedup 1.0000x reference)
"""Trainium2 Bass kernel for nn_CrossAttentionFusion.

Problem (hardcoded shapes): B=2, C1=64, C2=256, D=256, NH=8, HD=32, H=W=64,
n = H*W = 4096 tokens per batch image.

    xl = F_lidar tokens (B, n, C1); xc = F_cam tokens (B, n, C2)
    Q = xl@Wq^T, K = xc@Wk^T, V = xc@Wv^T  (per-head HD=32)
    attn = softmax(QK^T/sqrt(HD)); out = attn@V
    x = LN1(xl@Wres^T + out@Wo^T); x = LN2(x + FFN(x)); return (B, D, H, W)

Sharding: 8 cores, zero collectives. Core i handles batch b=i//4 and the
1024-token q-slice (i%4). K/V for the whole image are recomputed per core.

Attention inner loop (per d-group g of 4 heads, per 512-q block):
  Scores S^T (k-part, q-free) via 4 row-packed K=32 matmuls (all 4 heads of
  the group concurrently) into two [128,1024] PSUM tiles. Softmax exp is
  split across engines: heads 0,1 exact exp on ACT; heads 2,3 Schraudolph
  fast-exp on DVE (tensor_scalar mult+add into an int16 view of the bf16 e
  tile; bf16-bit-trick softmax validated at ~8e-4 model rel err). AV
  col-packed 4-wide accumulates over kc into one PSUM bank; a col-packed
  all-ones round accumulates softmax denominators onto exactly the 32
  partitions of their head's AV rows, so normalization is one elementwise
  PSUM multiply by reciprocal_approx_fast of the denominators. The PE
  stream is software-pipelined one k-chunk ahead (scores(kc+1) issued
  before AV(kc)) so ACT/DVE never wait on the PE.
"""

import numpy as np

B, C1, C2, D, NH, H, W = 2, 64, 256, 256, 8, 64, 64
HD = D // NH                 # 32
N_TOK = H * W                # 4096 tokens per image
N_CORES = 8
CORES_PER_B = N_CORES // B   # 4
NQ = N_TOK // CORES_PER_B    # 1024 q tokens per core
EPS = 1e-5
SCALE = HD ** -0.5
KC = N_TOK // 128            # 32 k-chunks
QT_TILES = NQ // 128         # 8 q-tiles of 128
F1 = 4 * D                   # 1024 FFN hidden

# Schraudolph fast-exp in bf16 bits: bits_i16 = s*SCALE*(128/ln2) + C2
FEXP_C1 = SCALE * 128.0 / np.log(2.0)
FEXP_C2 = 16252.0

_built = None


def _build():
    from contextlib import ExitStack

    import concourse.mybir as mybir
    import concourse.tile as tile
    from concourse import bacc
    from concourse.masks import make_identity

    F32 = mybir.dt.float32
    F32R = mybir.dt.float32r
    BF16 = mybir.dt.bfloat16
    I16 = mybir.dt.int16
    AF = mybir.ActivationFunctionType
    OP = mybir.AluOpType

    nc = bacc.Bacc(trn_type="TRN2", target_bir_lowering=False, debug=False,
                   num_devices=N_CORES)

    # ---- DRAM I/O ----
    xq = nc.dram_tensor("xq", [C1, NQ], BF16, kind="ExternalInput").ap()
    xc = nc.dram_tensor("xc", [C2, N_TOK], BF16, kind="ExternalInput").ap()
    wkt = nc.dram_tensor("wkt", [C2, D], BF16, kind="ExternalInput").ap()
    wvt = nc.dram_tensor("wvt", [C2, D], BF16, kind="ExternalInput").ap()
    wqt = nc.dram_tensor("wqt", [C1, D], BF16, kind="ExternalInput").ap()
    wrt = nc.dram_tensor("wrt", [C1, D], BF16, kind="ExternalInput").ap()
    wot = nc.dram_tensor("wot", [D, D], BF16, kind="ExternalInput").ap()
    w1t = nc.dram_tensor("w1t", [D, F1], BF16, kind="ExternalInput").ap()
    w2t = nc.dram_tensor("w2t", [F1, D], BF16, kind="ExternalInput").ap()
    g1 = nc.dram_tensor("g1", [D], F32, kind="ExternalInput").ap()
    b1 = nc.dram_tensor("b1", [D], F32, kind="ExternalInput").ap()
    g2 = nc.dram_tensor("g2", [D], F32, kind="ExternalInput").ap()
    b2 = nc.dram_tensor("b2", [D], F32, kind="ExternalInput").ap()
    bf1 = nc.dram_tensor("bf1", [F1], F32, kind="ExternalInput").ap()
    bf2 = nc.dram_tensor("bf2", [D], F32, kind="ExternalInput").ap()
    out = nc.dram_tensor("out", [NQ, D], F32, kind="ExternalOutput").ap()

    with tile.TileContext(nc) as tc, ExitStack() as ctx:
        # ---- persistent SBUF ----
        P = ctx.enter_context(tc.tile_pool(name="persist", bufs=1))

        xq_sb = P.tile([C1, NQ], BF16, name="xq_sb")
        wot_sb = [P.tile([128, D], BF16, name=f"wot{c}") for c in range(2)]
        w1t_sb = [P.tile([128, F1], BF16, name=f"w1t{c}") for c in range(2)]
        w2t_sb = P.tile([128, 8, D], BF16, name="w2t_sb")
        bf1_col = P.tile([128, 8], F32, name="bf1_col")
        kt_sb = [P.tile([128, N_TOK], BF16, name=f"kt{g}") for g in range(2)]
        v_sb = P.tile([128, KC, D], BF16, name="v_sb")
        qt_sb = [P.tile([128, NQ], BF16, name=f"qt{g}") for g in range(2)]
        resid_sb = P.tile([128, QT_TILES, D], F32, name="resid_sb")
        attn_sb = [P.tile([128, NQ], BF16, name=f"attn{g}") for g in range(2)]
        ones_sb = P.tile([128, HD], BF16, name="ones_sb")
        ident = P.tile([128, 128], F32, name="ident")
        eps_sb = P.tile([128, 1], F32, name="eps_sb")
        g1_bc = P.tile([128, D], F32, name="g1_bc")
        b1_bc = P.tile([128, D], F32, name="b1_bc")
        g2_bc = P.tile([128, D], F32, name="g2_bc")
        b2_bc = P.tile([128, D], F32, name="b2_bc")
        bf2_bc = P.tile([128, D], F32, name="bf2_bc")
        x1_sb = P.tile([128, QT_TILES, D], F32, name="x1_sb")
        x1t_sb = [P.tile([128, NQ], BF16, name=f"x1t{g}") for g in range(2)]
        hdn_sb = P.tile([128, 8, NQ], BF16, name="hdn_sb")

        nc.vector.memset(ones_sb, 1.0)
        nc.vector.memset(eps_sb, EPS)
        make_identity(nc, ident)

        def bcast_row(eng, dst, src_ap):
            # (n,) dram -> (128, n) sbuf, replicated on all partitions
            import concourse.bass as bass
            src = bass.AP(tensor=src_ap.tensor, offset=src_ap.offset,
                          ap=[[0, 128]] + src_ap.ap)
            eng.dma_start(dst, src)

        # critical-path DMAs on the SP queue: q-side, then xc (chunked).
        nc.sync.dma_start(xq_sb, xq)

        # =============== Phase A: projections ===============
        NTH = N_TOK // 2  # token-half for chunked xc arrival
        with tc.tile_pool(name="xc_pool", bufs=1) as XP, \
             tc.tile_pool(name="psA", bufs=2, space="PSUM") as psA:
            wqt_sb = XP.tile([C1, D], BF16, name="wqt_sb")
            wrt_sb = XP.tile([C1, D], BF16, name="wrt_sb")
            wkt_sb = [XP.tile([128, D], BF16, name=f"wkt{c}")
                      for c in range(2)]
            wvt_sb = [XP.tile([128, D], BF16, name=f"wvt{c}")
                      for c in range(2)]
            xc_sb = [XP.tile([128, N_TOK], BF16, name=f"xc{c}")
                     for c in range(2)]
            nc.sync.dma_start(wqt_sb, wqt)
            nc.sync.dma_start(wrt_sb, wrt)
            for c in range(2):
                nc.sync.dma_start(wkt_sb[c], wkt[128 * c:128 * (c + 1), :])
                nc.sync.dma_start(wvt_sb[c], wvt[128 * c:128 * (c + 1), :])
            # token-chunked arrival: both channel halves of tokens [0,2048)
            # first, so K/V projections start at ~half the xc DMA time.
            for th in range(2):
                for c in range(2):
                    nc.sync.dma_start(
                        xc_sb[c][:, NTH * th:NTH * (th + 1)],
                        xc[128 * c:128 * (c + 1), NTH * th:NTH * (th + 1)])
            # late-needed weights on the gpsimd SWDGE queue (keeps ACT free)
            for c in range(2):
                nc.gpsimd.dma_start(wot_sb[c], wot[128 * c:128 * (c + 1), :])
                nc.gpsimd.dma_start(w1t_sb[c], w1t[128 * c:128 * (c + 1), :])
            nc.gpsimd.dma_start(
                w2t_sb, w2t.rearrange("(a p) d -> p a d", p=128))
            nc.gpsimd.dma_start(bf1_col, bf1.rearrange("(a p) -> p a", p=128))
            bcast_row(nc.gpsimd, g1_bc, g1)
            bcast_row(nc.gpsimd, b1_bc, b1)
            bcast_row(nc.gpsimd, g2_bc, g2)
            bcast_row(nc.gpsimd, b2_bc, b2)
            bcast_row(nc.gpsimd, bf2_bc, bf2)

            # QT[d,q] = sum_c WqT[c,d] * xqT[c,q]  (only needs xq)
            for g in range(2):
                for qs in range(NQ // 512):
                    qp = psA.tile([128, 512], F32, name="qp")
                    nc.tensor.matmul(
                        qp, wqt_sb[:, 128 * g:128 * (g + 1)],
                        xq_sb[:, 512 * qs:512 * (qs + 1)],
                        start=True, stop=True)
                    nc.scalar.copy(
                        qt_sb[g][:, 512 * qs:512 * (qs + 1)], qp)
            # resid[q,d] = sum_c xqT[c,q] * WresT[c,d]
            for qt_i in range(QT_TILES):
                rp = psA.tile([128, D], F32, name="rp")
                nc.tensor.matmul(
                    rp, xq_sb[:, 128 * qt_i:128 * (qt_i + 1)],
                    wrt_sb, start=True, stop=True)
                nc.vector.tensor_copy(resid_sb[:, qt_i, :], rp)

            # KT[d,k] = sum_c WkT[c,d] * xcT[c,k]  (copies on ACT);
            # V[k,d] bf16 (copies on DVE), per token-half behind the DMA.
            for th in range(2):
                for g in range(2):
                    for ks in range(4):
                        kk = 4 * th + ks
                        kp = psA.tile([128, 512], F32, name="kp")
                        for c in range(2):
                            nc.tensor.matmul(
                                kp, wkt_sb[c][:, 128 * g:128 * (g + 1)],
                                xc_sb[c][:, 512 * kk:512 * (kk + 1)],
                                start=(c == 0), stop=(c == 1))
                        nc.scalar.copy(
                            kt_sb[g][:, 512 * kk:512 * (kk + 1)], kp)
                for ks in range(KC // 2):
                    kt_i = (KC // 2) * th + ks
                    vp = psA.tile([128, D], F32, name="vp")
                    for c in range(2):
                        nc.tensor.matmul(
                            vp, xc_sb[c][:, 128 * kt_i:128 * (kt_i + 1)],
                            wvt_sb[c], start=(c == 0), stop=(c == 1))
                    nc.vector.tensor_copy(v_sb[:, kt_i, :], vp)

        # =============== Phase B: attention ===============
        with tc.tile_pool(name="scps", bufs=3, space="PSUM") as scps, \
             tc.tile_pool(name="avps", bufs=1, space="PSUM") as avps, \
             tc.tile_pool(name="epool", bufs=3) as epool, \
             tc.tile_pool(name="nrm", bufs=2) as nrm:
            for qc in range(2):
                qs = slice(512 * qc, 512 * (qc + 1))
                for g in range(2):
                    av = avps.tile([128, 512], F32, name="av")
                    ao = avps.tile([128, 512], F32, name="ao")

                    def scores(kc):
                        ks = slice(128 * kc, 128 * (kc + 1))
                        sc = [scps.tile([128, 1024], F32, name="sc")
                              for i in range(2)]
                        for h in range(4):
                            p = 32 * h
                            nc.tensor.matmul(
                                sc[h // 2][:, 512 * (h % 2):512 * (h % 2 + 1)],
                                kt_sb[g][p:p + 32, ks],
                                qt_sb[g][p:p + 32, qs],
                                start=True, stop=True, tile_position=(p, 0))
                        e = epool.tile([128, 4 * 512], BF16, name="e")
                        # heads 0,1: exact exp on ACT
                        nc.scalar.activation(
                            e[:, 0:1024], sc[0], AF.Exp, scale=SCALE)
                        # heads 2,3: Schraudolph fast-exp on DVE
                        nc.vector.tensor_scalar(
                            out=e[:, 1024:2048].bitcast(I16), in0=sc[1],
                            scalar1=float(FEXP_C1), scalar2=float(FEXP_C2),
                            op0=OP.mult, op1=OP.add)
                        return e

                    e_cur = scores(0)
                    for kc in range(KC):
                        e_next = scores(kc + 1) if kc + 1 < KC else None
                        st, sp = (kc == 0), (kc == KC - 1)
                        for h in range(4):
                            p = 32 * h
                            es = e_cur[:, 512 * h:512 * (h + 1)]
                            nc.tensor.matmul(
                                av[p:p + 32, :],
                                v_sb[:, kc, HD * (4 * g + h):HD * (4 * g + h) + HD],
                                es, start=st, stop=sp,
                                tile_position=(0, p), skip_group_check=True)
                        for h in range(4):
                            p = 32 * h
                            es = e_cur[:, 512 * h:512 * (h + 1)]
                            nc.tensor.matmul(
                                ao[p:p + 32, :], ones_sb, es,
                                start=st, stop=sp,
                                tile_position=(0, p), skip_group_check=True)
                        e_cur = e_next

                    rec = nrm.tile([128, 512], F32, name="rec")
                    nc.vector.reciprocal_approx_fast(out=rec, in_=ao)
                    nc.vector.tensor_mul(attn_sb[g][:, qs], av, rec)

        # =============== Phase C: Wo + LN1 + transpose ===============
        with tc.tile_pool(name="psC", bufs=1, space="PSUM") as psC, \
             tc.tile_pool(name="tpps", bufs=2, space="PSUM") as tpps, \
             tc.tile_pool(name="psD", bufs=1, space="PSUM") as psD, \
             tc.tile_pool(name="lnp", bufs=4) as lnp, \
             tc.tile_pool(name="lnagg", bufs=1) as lnagg:
            mv_all = lnagg.tile([128, QT_TILES, 2], F32, name="mv_all")
            rstd_all = lnagg.tile([128, QT_TILES], F32, name="rstd_all")
            xp_all = lnagg.tile([128, QT_TILES, D], F32, name="xp_all")
            for qt_i in range(QT_TILES):
                pp = psC.tile([128, D], F32, name="pp")
                ts = slice(128 * qt_i, 128 * (qt_i + 1))
                for g in range(2):
                    nc.tensor.matmul(pp, attn_sb[g][:, ts], wot_sb[g],
                                     start=(g == 0), stop=(g == 1))
                xp = xp_all[:, qt_i, :]
                nc.vector.tensor_add(xp, pp, resid_sb[:, qt_i, :])
                stats = lnp.tile([128, 6], F32, name="stats")
                nc.vector.bn_stats(out=stats, in_=xp)
                nc.vector.bn_aggr(out=mv_all[:, qt_i, :], in_=stats)
            # batched rstd for all 8 tiles: one sqrt + one fast reciprocal
            sq = lnagg.tile([128, QT_TILES], F32, name="sq")
            nc.scalar.activation(sq, mv_all[:, :, 1], AF.Sqrt, bias=eps_sb)
            nc.vector.reciprocal_approx_fast(out=rstd_all, in_=sq)
            for qt_i in range(QT_TILES):
                ts = slice(128 * qt_i, 128 * (qt_i + 1))
                x1s = x1_sb[:, qt_i, :]
                nc.vector.tensor_scalar(
                    out=x1s, in0=xp_all[:, qt_i, :],
                    scalar1=mv_all[:, qt_i, 0:1],
                    scalar2=rstd_all[:, qt_i:qt_i + 1],
                    op0=OP.subtract, op1=OP.mult)
                nc.vector.tensor_mul(x1s, x1s, g1_bc)
                nc.vector.tensor_add(x1s, x1s, b1_bc)
                for dc in range(2):
                    tp = tpps.tile([128, 128], F32, name="tp")
                    nc.tensor.transpose(
                        tp, x1_sb[:, qt_i, 128 * dc:128 * (dc + 1)], ident)
                    nc.scalar.copy(x1t_sb[dc][:, ts], tp)

            # =============== Phase D: FFN + LN2 ===============
            # hdn^T[f,q] = relu(sum_d W1T[d,f] x1T[d,q] + bf1[f]), relu
            # split ACT/DVE; FFN2 accumulation for tiles 0-3 rides one fc
            # step behind FFN1 in the same PE stream.
            fp_half = [psD.tile([128, D], F32, name=f"fph{i}")
                       for i in range(4)]
            for fc in range(8):
                for qcb in range(NQ // 512):
                    qsl = slice(512 * qcb, 512 * (qcb + 1))
                    hp_ = psC.tile([128, 512], F32, name="hp_")
                    for dc in range(2):
                        nc.tensor.matmul(
                            hp_, w1t_sb[dc][:, 128 * fc:128 * (fc + 1)],
                            x1t_sb[dc][:, qsl], start=(dc == 0), stop=(dc == 1))
                    if fc % 2 == 0:
                        nc.scalar.activation(
                            hdn_sb[:, fc, qsl], hp_, AF.Relu,
                            bias=bf1_col[:, fc:fc + 1])
                    else:
                        nc.vector.tensor_scalar(
                            out=hdn_sb[:, fc, qsl], in0=hp_,
                            scalar1=bf1_col[:, fc:fc + 1], scalar2=0.0,
                            op0=OP.add, op1=OP.max)
                for qt_i in range(4):
                    ts = slice(128 * qt_i, 128 * (qt_i + 1))
                    nc.tensor.matmul(
                        fp_half[qt_i], hdn_sb[:, fc, ts], w2t_sb[:, fc, :],
                        start=(fc == 0), stop=(fc == 7),
                        skip_group_check=True)
            # ffn[q,d] = sum_f hdnT[f,q] W2T[f,d]; x2 = LN2(x1+ffn+bf2)
            mv2_all = lnagg.tile([128, QT_TILES, 2], F32, name="mv2_all")
            rstd2_all = lnagg.tile([128, QT_TILES], F32, name="rstd2_all")
            xp2_all = lnagg.tile([128, QT_TILES, D], F32, name="xp2_all")
            for qt_i in range(QT_TILES):
                ts = slice(128 * qt_i, 128 * (qt_i + 1))
                if qt_i < 4:
                    fp = fp_half[qt_i]
                else:
                    if qt_i == 4:
                        fp_half = [psD.tile([128, D], F32, name=f"fph{i}")
                                   for i in range(4)]
                    fp = fp_half[qt_i - 4]
                    for fc in range(8):
                        nc.tensor.matmul(
                            fp, hdn_sb[:, fc, ts], w2t_sb[:, fc, :],
                            start=(fc == 0), stop=(fc == 7),
                            skip_group_check=True)
                xp2 = xp2_all[:, qt_i, :]
                nc.vector.tensor_add(xp2, fp, x1_sb[:, qt_i, :])
                nc.vector.tensor_add(xp2, xp2, bf2_bc)
                stats2 = lnp.tile([128, 6], F32, name="stats2")
                nc.vector.bn_stats(out=stats2, in_=xp2)
                nc.vector.bn_aggr(out=mv2_all[:, qt_i, :], in_=stats2)
            sq2 = lnagg.tile([128, QT_TILES], F32, name="sq2")
            nc.scalar.activation(sq2, mv2_all[:, :, 1], AF.Sqrt, bias=eps_sb)
            nc.vector.reciprocal_approx_fast(out=rstd2_all, in_=sq2)
            for qt_i in range(QT_TILES):
                ts = slice(128 * qt_i, 128 * (qt_i + 1))
                xo = lnp.tile([128, D], F32, name="xo")
                nc.vector.tensor_scalar(
                    out=xo, in0=xp2_all[:, qt_i, :],
                    scalar1=mv2_all[:, qt_i, 0:1],
                    scalar2=rstd2_all[:, qt_i:qt_i + 1],
                    op0=OP.subtract, op1=OP.mult)
                nc.vector.tensor_mul(xo, xo, g2_bc)
                nc.vector.tensor_add(xo, xo, b2_bc)
                nc.sync.dma_start(out[ts, :], xo)

    nc.compile()
    return nc


def _get_nc():
    global _built
    if _built is None:
        _built = _build()
    return _built


def _make_in_maps(inputs):
    f32 = np.float32
    F_lidar = np.ascontiguousarray(inputs["F_lidar"], dtype=f32)
    F_cam = np.ascontiguousarray(inputs["F_cam"], dtype=f32)
    import ml_dtypes
    bf16 = ml_dtypes.bfloat16
    common = {
        "wkt": np.ascontiguousarray(inputs["Wk"].T).astype(bf16),
        "wvt": np.ascontiguousarray(inputs["Wv"].T).astype(bf16),
        "wqt": np.ascontiguousarray(inputs["Wq"].T).astype(bf16),
        "wrt": np.ascontiguousarray(inputs["Wres"].T).astype(bf16),
        "wot": np.ascontiguousarray(inputs["Wo"].T).astype(bf16),
        "w1t": np.ascontiguousarray(inputs["W1"].T).astype(bf16),
        "w2t": np.ascontiguousarray(inputs["W2"].T).astype(bf16),
        "g1": np.asarray(inputs["g1"], f32), "b1": np.asarray(inputs["b1"], f32),
        "g2": np.asarray(inputs["g2"], f32), "b2": np.asarray(inputs["b2"], f32),
        "bf1": np.asarray(inputs["bf1"], f32),
        "bf2": np.asarray(inputs["bf2"], f32),
    }
    in_maps = []
    for c in range(N_CORES):
        b, s = c // CORES_PER_B, (c % CORES_PER_B) * NQ
        m = dict(common)
        m["xq"] = np.ascontiguousarray(
            F_lidar[b].reshape(C1, N_TOK)[:, s:s + NQ]).astype(bf16)
        m["xc"] = np.ascontiguousarray(
            F_cam[b].reshape(C2, N_TOK)).astype(bf16)
        in_maps.append(m)
    return in_maps


def kernel(**inputs):
    from concourse.bass_utils import run_bass_kernel_spmd

    nc = _get_nc()
    in_maps = _make_in_maps(inputs)
    res = run_bass_kernel_spmd(nc, in_maps, list(range(N_CORES)))
    out = np.empty((B, D, N_TOK), dtype=np.float32)
    for c in range(N_CORES):
        b, s = c // CORES_PER_B, (c % CORES_PER_B) * NQ
        out[b, :, s:s + NQ] = res.results[c]["out"].T
    return out.reshape(B, D, H, W)


# revision 17
# speedup vs baseline: 1.0003x; 1.0003x over previous
"""Trainium2 Bass kernel for nn_CrossAttentionFusion.

Problem (hardcoded shapes): B=2, C1=64, C2=256, D=256, NH=8, HD=32, H=W=64,
n = H*W = 4096 tokens per batch image.

    xl = F_lidar tokens (B, n, C1); xc = F_cam tokens (B, n, C2)
    Q = xl@Wq^T, K = xc@Wk^T, V = xc@Wv^T  (per-head HD=32)
    attn = softmax(QK^T/sqrt(HD)); out = attn@V
    x = LN1(xl@Wres^T + out@Wo^T); x = LN2(x + FFN(x)); return (B, D, H, W)

Sharding: 8 cores, zero collectives. Core i handles batch b=i//4 and the
1024-token q-slice (i%4). K/V for the whole image are recomputed per core.

Attention inner loop (per d-group g of 4 heads, per 512-q block):
  Scores S^T (k-part, q-free) via 4 row-packed K=32 matmuls (all 4 heads of
  the group concurrently) into two [128,1024] PSUM tiles. Softmax exp is
  split across engines: heads 0,1 exact exp on ACT; heads 2,3 Schraudolph
  fast-exp on DVE (tensor_scalar mult+add into an int16 view of the bf16 e
  tile; bf16-bit-trick softmax validated at ~8e-4 model rel err). AV
  col-packed 4-wide accumulates over kc into one PSUM bank; a col-packed
  all-ones round accumulates softmax denominators onto exactly the 32
  partitions of their head's AV rows, so normalization is one elementwise
  PSUM multiply by reciprocal_approx_fast of the denominators. The PE
  stream is software-pipelined one k-chunk ahead (scores(kc+1) issued
  before AV(kc)) so ACT/DVE never wait on the PE.
"""

import numpy as np

B, C1, C2, D, NH, H, W = 2, 64, 256, 256, 8, 64, 64
HD = D // NH                 # 32
N_TOK = H * W                # 4096 tokens per image
N_CORES = 8
CORES_PER_B = N_CORES // B   # 4
NQ = N_TOK // CORES_PER_B    # 1024 q tokens per core
EPS = 1e-5
SCALE = HD ** -0.5
KC = N_TOK // 128            # 32 k-chunks
QT_TILES = NQ // 128         # 8 q-tiles of 128
F1 = 4 * D                   # 1024 FFN hidden

# Schraudolph fast-exp in bf16 bits: bits_i16 = s*SCALE*(128/ln2) + C2
FEXP_C1 = SCALE * 128.0 / np.log(2.0)
FEXP_C2 = 16252.0

_built = None


def _build():
    from contextlib import ExitStack

    import concourse.mybir as mybir
    import concourse.tile as tile
    from concourse import bacc
    from concourse.masks import make_identity

    F32 = mybir.dt.float32
    F32R = mybir.dt.float32r
    BF16 = mybir.dt.bfloat16
    I16 = mybir.dt.int16
    AF = mybir.ActivationFunctionType
    OP = mybir.AluOpType

    nc = bacc.Bacc(trn_type="TRN2", target_bir_lowering=False, debug=False,
                   num_devices=N_CORES)

    # ---- DRAM I/O ----
    xq = nc.dram_tensor("xq", [C1, NQ], BF16, kind="ExternalInput").ap()
    xc = nc.dram_tensor("xc", [C2, N_TOK], BF16, kind="ExternalInput").ap()
    wkt = nc.dram_tensor("wkt", [C2, D], BF16, kind="ExternalInput").ap()
    wvt = nc.dram_tensor("wvt", [C2, D], BF16, kind="ExternalInput").ap()
    wqt = nc.dram_tensor("wqt", [C1, D], BF16, kind="ExternalInput").ap()
    wrt = nc.dram_tensor("wrt", [C1, D], BF16, kind="ExternalInput").ap()
    wot = nc.dram_tensor("wot", [D, D], BF16, kind="ExternalInput").ap()
    w1t = nc.dram_tensor("w1t", [D, F1], BF16, kind="ExternalInput").ap()
    w2t = nc.dram_tensor("w2t", [F1, D], BF16, kind="ExternalInput").ap()
    g1 = nc.dram_tensor("g1", [D], F32, kind="ExternalInput").ap()
    b1 = nc.dram_tensor("b1", [D], F32, kind="ExternalInput").ap()
    g2 = nc.dram_tensor("g2", [D], F32, kind="ExternalInput").ap()
    b2 = nc.dram_tensor("b2", [D], F32, kind="ExternalInput").ap()
    bf1 = nc.dram_tensor("bf1", [F1], F32, kind="ExternalInput").ap()
    bf2 = nc.dram_tensor("bf2", [D], F32, kind="ExternalInput").ap()
    out = nc.dram_tensor("out", [NQ, D], F32, kind="ExternalOutput").ap()

    with tile.TileContext(nc) as tc, ExitStack() as ctx:
        # ---- persistent SBUF ----
        P = ctx.enter_context(tc.tile_pool(name="persist", bufs=1))

        xq_sb = P.tile([C1, NQ], BF16, name="xq_sb")
        wot_sb = [P.tile([128, D], BF16, name=f"wot{c}") for c in range(2)]
        w1t_sb = [P.tile([128, F1], BF16, name=f"w1t{c}") for c in range(2)]
        w2t_sb = P.tile([128, 8, D], BF16, name="w2t_sb")
        bf1_col = P.tile([128, 8], F32, name="bf1_col")
        kt_sb = [P.tile([128, N_TOK], BF16, name=f"kt{g}") for g in range(2)]
        v_sb = P.tile([128, KC, D], BF16, name="v_sb")
        qt_sb = [P.tile([128, NQ], BF16, name=f"qt{g}") for g in range(2)]
        resid_sb = P.tile([128, QT_TILES, D], F32, name="resid_sb")
        attn_sb = [P.tile([128, NQ], BF16, name=f"attn{g}") for g in range(2)]
        ones_sb = P.tile([128, HD], BF16, name="ones_sb")
        ident = P.tile([128, 128], F32, name="ident")
        eps_sb = P.tile([128, 1], F32, name="eps_sb")
        g1_bc = P.tile([128, D], F32, name="g1_bc")
        b1_bc = P.tile([128, D], F32, name="b1_bc")
        g2_bc = P.tile([128, D], F32, name="g2_bc")
        b2_bc = P.tile([128, D], F32, name="b2_bc")
        bf2_bc = P.tile([128, D], F32, name="bf2_bc")
        x1_sb = P.tile([128, QT_TILES, D], F32, name="x1_sb")
        x1t_sb = [P.tile([128, NQ], BF16, name=f"x1t{g}") for g in range(2)]
        hdn_sb = P.tile([128, 8, NQ], BF16, name="hdn_sb")

        nc.vector.memset(ones_sb, 1.0)
        nc.vector.memset(eps_sb, EPS)
        make_identity(nc, ident)

        def bcast_row(eng, dst, src_ap):
            # (n,) dram -> (128, n) sbuf, replicated on all partitions
            import concourse.bass as bass
            src = bass.AP(tensor=src_ap.tensor, offset=src_ap.offset,
                          ap=[[0, 128]] + src_ap.ap)
            eng.dma_start(dst, src)

        # critical-path DMAs on the SP queue: q-side, then xc (chunked).
        nc.sync.dma_start(xq_sb, xq)

        # =============== Phase A: projections ===============
        NTH = N_TOK // 2  # token-half for chunked xc arrival
        with tc.tile_pool(name="xc_pool", bufs=1) as XP, \
             tc.tile_pool(name="psA", bufs=2, space="PSUM") as psA:
            wqt_sb = XP.tile([C1, D], BF16, name="wqt_sb")
            wrt_sb = XP.tile([C1, D], BF16, name="wrt_sb")
            wkt_sb = [XP.tile([128, D], BF16, name=f"wkt{c}")
                      for c in range(2)]
            wvt_sb = [XP.tile([128, D], BF16, name=f"wvt{c}")
                      for c in range(2)]
            xc_sb = [XP.tile([128, N_TOK], BF16, name=f"xc{c}")
                     for c in range(2)]
            nc.sync.dma_start(wqt_sb, wqt)
            nc.sync.dma_start(wrt_sb, wrt)
            for c in range(2):
                nc.sync.dma_start(wkt_sb[c], wkt[128 * c:128 * (c + 1), :])
                nc.sync.dma_start(wvt_sb[c], wvt[128 * c:128 * (c + 1), :])
            # token-chunked arrival: both channel halves of tokens [0,2048)
            # first, so K/V projections start at ~half the xc DMA time.
            for th in range(2):
                for c in range(2):
                    nc.sync.dma_start(
                        xc_sb[c][:, NTH * th:NTH * (th + 1)],
                        xc[128 * c:128 * (c + 1), NTH * th:NTH * (th + 1)])
            # late-needed weights on the gpsimd SWDGE queue (keeps ACT free)
            for c in range(2):
                nc.gpsimd.dma_start(wot_sb[c], wot[128 * c:128 * (c + 1), :])
                nc.gpsimd.dma_start(w1t_sb[c], w1t[128 * c:128 * (c + 1), :])
            nc.gpsimd.dma_start(
                w2t_sb, w2t.rearrange("(a p) d -> p a d", p=128))
            nc.gpsimd.dma_start(bf1_col, bf1.rearrange("(a p) -> p a", p=128))
            bcast_row(nc.gpsimd, g1_bc, g1)
            bcast_row(nc.gpsimd, b1_bc, b1)
            bcast_row(nc.gpsimd, g2_bc, g2)
            bcast_row(nc.gpsimd, b2_bc, b2)
            bcast_row(nc.gpsimd, bf2_bc, bf2)

            # QT[d,q] = sum_c WqT[c,d] * xqT[c,q]  (only needs xq)
            for g in range(2):
                for qs in range(NQ // 512):
                    qp = psA.tile([128, 512], F32, name="qp")
                    nc.tensor.matmul(
                        qp, wqt_sb[:, 128 * g:128 * (g + 1)],
                        xq_sb[:, 512 * qs:512 * (qs + 1)],
                        start=True, stop=True)
                    nc.scalar.copy(
                        qt_sb[g][:, 512 * qs:512 * (qs + 1)], qp)
            # resid[q,d] = sum_c xqT[c,q] * WresT[c,d]
            for qt_i in range(QT_TILES):
                rp = psA.tile([128, D], F32, name="rp")
                nc.tensor.matmul(
                    rp, xq_sb[:, 128 * qt_i:128 * (qt_i + 1)],
                    wrt_sb, start=True, stop=True)
                nc.vector.tensor_copy(resid_sb[:, qt_i, :], rp)

            # KT[d,k] = sum_c WkT[c,d] * xcT[c,k];  V[k,d] bf16.
            # kp/vp interleaved and their PSUM->SBUF copies alternate between
            # ACT and DVE so both engine queues drain concurrently.
            for th in range(2):
                for ks in range(4):
                    for g in range(2):
                        kk = 4 * th + ks
                        kp = psA.tile([128, 512], F32, name="kp")
                        for c in range(2):
                            nc.tensor.matmul(
                                kp, wkt_sb[c][:, 128 * g:128 * (g + 1)],
                                xc_sb[c][:, 512 * kk:512 * (kk + 1)],
                                start=(c == 0), stop=(c == 1))
                        dst = kt_sb[g][:, 512 * kk:512 * (kk + 1)]
                        if g == 0:
                            nc.scalar.copy(dst, kp)
                        else:
                            nc.vector.tensor_copy(dst, kp)
                    for vi in range(4):
                        kt_i = 16 * th + 4 * ks + vi
                        vp = psA.tile([128, D], F32, name="vp")
                        for c in range(2):
                            nc.tensor.matmul(
                                vp, xc_sb[c][:, 128 * kt_i:128 * (kt_i + 1)],
                                wvt_sb[c], start=(c == 0), stop=(c == 1))
                        if vi % 2 == 0:
                            nc.vector.tensor_copy(v_sb[:, kt_i, :], vp)
                        else:
                            nc.scalar.copy(v_sb[:, kt_i, :], vp)

        # =============== Phase B: attention ===============
        with tc.tile_pool(name="scps", bufs=3, space="PSUM") as scps, \
             tc.tile_pool(name="avps", bufs=1, space="PSUM") as avps, \
             tc.tile_pool(name="epool", bufs=3) as epool, \
             tc.tile_pool(name="nrm", bufs=2) as nrm:
            for qc in range(2):
                qs = slice(512 * qc, 512 * (qc + 1))
                for g in range(2):
                    av = avps.tile([128, 512], F32, name="av")
                    ao = avps.tile([128, 512], F32, name="ao")

                    def scores(kc):
                        ks = slice(128 * kc, 128 * (kc + 1))
                        sc = [scps.tile([128, 1024], F32, name="sc")
                              for i in range(2)]
                        for h in range(4):
                            p = 32 * h
                            nc.tensor.matmul(
                                sc[h // 2][:, 512 * (h % 2):512 * (h % 2 + 1)],
                                kt_sb[g][p:p + 32, ks],
                                qt_sb[g][p:p + 32, qs],
                                start=True, stop=True, tile_position=(p, 0))
                        e = epool.tile([128, 4 * 512], BF16, name="e")
                        # heads 0,1: exact exp on ACT
                        nc.scalar.activation(
                            e[:, 0:1024], sc[0], AF.Exp, scale=SCALE)
                        # heads 2,3: Schraudolph fast-exp on DVE
                        nc.vector.tensor_scalar(
                            out=e[:, 1024:2048].bitcast(I16), in0=sc[1],
                            scalar1=float(FEXP_C1), scalar2=float(FEXP_C2),
                            op0=OP.mult, op1=OP.add)
                        return e

                    e_cur = scores(0)
                    for kc in range(KC):
                        e_next = scores(kc + 1) if kc + 1 < KC else None
                        st, sp = (kc == 0), (kc == KC - 1)
                        for h in range(4):
                            p = 32 * h
                            es = e_cur[:, 512 * h:512 * (h + 1)]
                            nc.tensor.matmul(
                                av[p:p + 32, :],
                                v_sb[:, kc, HD * (4 * g + h):HD * (4 * g + h) + HD],
                                es, start=st, stop=sp,
                                tile_position=(0, p), skip_group_check=True)
                        for h in range(4):
                            p = 32 * h
                            es = e_cur[:, 512 * h:512 * (h + 1)]
                            nc.tensor.matmul(
                                ao[p:p + 32, :], ones_sb, es,
                                start=st, stop=sp,
                                tile_position=(0, p), skip_group_check=True)
                        e_cur = e_next

                    rec = nrm.tile([128, 512], F32, name="rec")
                    nc.vector.reciprocal_approx_fast(out=rec, in_=ao)
                    nc.vector.tensor_mul(attn_sb[g][:, qs], av, rec)

        # =============== Phase C: Wo + LN1 + transpose ===============
        with tc.tile_pool(name="psC", bufs=1, space="PSUM") as psC, \
             tc.tile_pool(name="tpps", bufs=2, space="PSUM") as tpps, \
             tc.tile_pool(name="psD", bufs=1, space="PSUM") as psD, \
             tc.tile_pool(name="lnp", bufs=4) as lnp, \
             tc.tile_pool(name="lnagg", bufs=1) as lnagg:
            mv_all = lnagg.tile([128, QT_TILES, 2], F32, name="mv_all")
            rstd_all = lnagg.tile([128, QT_TILES], F32, name="rstd_all")
            xp_all = lnagg.tile([128, QT_TILES, D], F32, name="xp_all")
            for qt_i in range(QT_TILES):
                pp = psC.tile([128, D], F32, name="pp")
                ts = slice(128 * qt_i, 128 * (qt_i + 1))
                for g in range(2):
                    nc.tensor.matmul(pp, attn_sb[g][:, ts], wot_sb[g],
                                     start=(g == 0), stop=(g == 1))
                xp = xp_all[:, qt_i, :]
                nc.vector.tensor_add(xp, pp, resid_sb[:, qt_i, :])
                stats = lnp.tile([128, 6], F32, name="stats")
                nc.vector.bn_stats(out=stats, in_=xp)
                nc.vector.bn_aggr(out=mv_all[:, qt_i, :], in_=stats)
            # batched rstd for all 8 tiles: one sqrt + one fast reciprocal
            sq = lnagg.tile([128, QT_TILES], F32, name="sq")
            nc.scalar.activation(sq, mv_all[:, :, 1], AF.Sqrt, bias=eps_sb)
            nc.vector.reciprocal_approx_fast(out=rstd_all, in_=sq)
            for qt_i in range(QT_TILES):
                ts = slice(128 * qt_i, 128 * (qt_i + 1))
                x1s = x1_sb[:, qt_i, :]
                nc.vector.tensor_scalar(
                    out=x1s, in0=xp_all[:, qt_i, :],
                    scalar1=mv_all[:, qt_i, 0:1],
                    scalar2=rstd_all[:, qt_i:qt_i + 1],
                    op0=OP.subtract, op1=OP.mult)
                nc.vector.tensor_mul(x1s, x1s, g1_bc)
                nc.vector.tensor_add(x1s, x1s, b1_bc)
                for dc in range(2):
                    tp = tpps.tile([128, 128], F32, name="tp")
                    nc.tensor.transpose(
                        tp, x1_sb[:, qt_i, 128 * dc:128 * (dc + 1)], ident)
                    nc.scalar.copy(x1t_sb[dc][:, ts], tp)

            # =============== Phase D: FFN + LN2 ===============
            # hdn^T[f,q] = relu(sum_d W1T[d,f] x1T[d,q] + bf1[f]), relu
            # split ACT/DVE; FFN2 accumulation for tiles 0-3 rides one fc
            # step behind FFN1 in the same PE stream.
            fp_half = [psD.tile([128, D], F32, name=f"fph{i}")
                       for i in range(4)]
            for fc in range(8):
                for qcb in range(NQ // 512):
                    qsl = slice(512 * qcb, 512 * (qcb + 1))
                    hp_ = psC.tile([128, 512], F32, name="hp_")
                    for dc in range(2):
                        nc.tensor.matmul(
                            hp_, w1t_sb[dc][:, 128 * fc:128 * (fc + 1)],
                            x1t_sb[dc][:, qsl], start=(dc == 0), stop=(dc == 1))
                    if fc % 2 == 0:
                        nc.scalar.activation(
                            hdn_sb[:, fc, qsl], hp_, AF.Relu,
                            bias=bf1_col[:, fc:fc + 1])
                    else:
                        nc.vector.tensor_scalar(
                            out=hdn_sb[:, fc, qsl], in0=hp_,
                            scalar1=bf1_col[:, fc:fc + 1], scalar2=0.0,
                            op0=OP.add, op1=OP.max)
                if fc >= 2:
                    for qt_i in range(4):
                        ts = slice(128 * qt_i, 128 * (qt_i + 1))
                        nc.tensor.matmul(
                            fp_half[qt_i], hdn_sb[:, fc - 2, ts],
                            w2t_sb[:, fc - 2, :],
                            start=(fc == 2), stop=False,
                            skip_group_check=True)
            for fc in range(6, 8):
                for qt_i in range(4):
                    ts = slice(128 * qt_i, 128 * (qt_i + 1))
                    nc.tensor.matmul(
                        fp_half[qt_i], hdn_sb[:, fc, ts], w2t_sb[:, fc, :],
                        start=False, stop=(fc == 7),
                        skip_group_check=True)
            # ffn[q,d] = sum_f hdnT[f,q] W2T[f,d]; x2 = LN2(x1+ffn+bf2)
            mv2_all = lnagg.tile([128, QT_TILES, 2], F32, name="mv2_all")
            rstd2_all = lnagg.tile([128, QT_TILES], F32, name="rstd2_all")
            xp2_all = lnagg.tile([128, QT_TILES, D], F32, name="xp2_all")
            for qt_i in range(QT_TILES):
                ts = slice(128 * qt_i, 128 * (qt_i + 1))
                if qt_i < 4:
                    fp = fp_half[qt_i]
                else:
                    if qt_i == 4:
                        fp_half = [psD.tile([128, D], F32, name=f"fph{i}")
                                   for i in range(4)]
                    fp = fp_half[qt_i - 4]
                    for fc in range(8):
                        nc.tensor.matmul(
                            fp, hdn_sb[:, fc, ts], w2t_sb[:, fc, :],
                            start=(fc == 0), stop=(fc == 7),
                            skip_group_check=True)
                xp2 = xp2_all[:, qt_i, :]
                nc.vector.tensor_add(xp2, fp, x1_sb[:, qt_i, :])
                nc.vector.tensor_add(xp2, xp2, bf2_bc)
                stats2 = lnp.tile([128, 6], F32, name="stats2")
                nc.vector.bn_stats(out=stats2, in_=xp2)
                nc.vector.bn_aggr(out=mv2_all[:, qt_i, :], in_=stats2)
            sq2 = lnagg.tile([128, QT_TILES], F32, name="sq2")
            nc.scalar.activation(sq2, mv2_all[:, :, 1], AF.Sqrt, bias=eps_sb)
            nc.vector.reciprocal_approx_fast(out=rstd2_all, in_=sq2)
            for qt_i in range(QT_TILES):
                ts = slice(128 * qt_i, 128 * (qt_i + 1))
                xo = lnp.tile([128, D], F32, name="xo")
                nc.vector.tensor_scalar(
                    out=xo, in0=xp2_all[:, qt_i, :],
                    scalar1=mv2_all[:, qt_i, 0:1],
                    scalar2=rstd2_all[:, qt_i:qt_i + 1],
                    op0=OP.subtract, op1=OP.mult)
                nc.vector.tensor_mul(xo, xo, g2_bc)
                nc.vector.tensor_add(xo, xo, b2_bc)
                nc.sync.dma_start(out[ts, :], xo)

    nc.compile()
    return nc


def _get_nc():
    global _built
    if _built is None:
        _built = _build()
    return _built


def _make_in_maps(inputs):
    f32 = np.float32
    F_lidar = np.ascontiguousarray(inputs["F_lidar"], dtype=f32)
    F_cam = np.ascontiguousarray(inputs["F_cam"], dtype=f32)
    import ml_dtypes
    bf16 = ml_dtypes.bfloat16
    common = {
        "wkt": np.ascontiguousarray(inputs["Wk"].T).astype(bf16),
        "wvt": np.ascontiguousarray(inputs["Wv"].T).astype(bf16),
        "wqt": np.ascontiguousarray(inputs["Wq"].T).astype(bf16),
        "wrt": np.ascontiguousarray(inputs["Wres"].T).astype(bf16),
        "wot": np.ascontiguousarray(inputs["Wo"].T).astype(bf16),
        "w1t": np.ascontiguousarray(inputs["W1"].T).astype(bf16),
        "w2t": np.ascontiguousarray(inputs["W2"].T).astype(bf16),
        "g1": np.asarray(inputs["g1"], f32), "b1": np.asarray(inputs["b1"], f32),
        "g2": np.asarray(inputs["g2"], f32), "b2": np.asarray(inputs["b2"], f32),
        "bf1": np.asarray(inputs["bf1"], f32),
        "bf2": np.asarray(inputs["bf2"], f32),
    }
    in_maps = []
    for c in range(N_CORES):
        b, s = c // CORES_PER_B, (c % CORES_PER_B) * NQ
        m = dict(common)
        m["xq"] = np.ascontiguousarray(
            F_lidar[b].reshape(C1, N_TOK)[:, s:s + NQ]).astype(bf16)
        m["xc"] = np.ascontiguousarray(
            F_cam[b].reshape(C2, N_TOK)).astype(bf16)
        in_maps.append(m)
    return in_maps


def kernel(**inputs):
    from concourse.bass_utils import run_bass_kernel_spmd

    nc = _get_nc()
    in_maps = _make_in_maps(inputs)
    res = run_bass_kernel_spmd(nc, in_maps, list(range(N_CORES)))
    out = np.empty((B, D, N_TOK), dtype=np.float32)
    for c in range(N_CORES):
        b, s = c // CORES_PER_B, (c % CORES_PER_B) * NQ
        out[b, :, s:s + NQ] = res.results[c]["out"].T
    return out.reshape(B, D, H, W)


# revision 18
# speedup vs baseline: 1.0569x; 1.0566x over previous
"""Trainium2 Bass kernel for nn_CrossAttentionFusion.

Problem (hardcoded shapes): B=2, C1=64, C2=256, D=256, NH=8, HD=32, H=W=64,
n = H*W = 4096 tokens per batch image.

    xl = F_lidar tokens (B, n, C1); xc = F_cam tokens (B, n, C2)
    Q = xl@Wq^T, K = xc@Wk^T, V = xc@Wv^T  (per-head HD=32)
    attn = softmax(QK^T/sqrt(HD)); out = attn@V
    x = LN1(xl@Wres^T + out@Wo^T); x = LN2(x + FFN(x)); return (B, D, H, W)

Sharding: 8 cores, zero collectives. Core i handles batch b=i//4 and the
1024-token q-slice (i%4). K/V for the whole image are recomputed per core.

Attention inner loop (per d-group g of 4 heads, per 512-q block):
  Scores S^T (k-part, q-free) via 4 row-packed K=32 matmuls (all 4 heads of
  the group concurrently) into two [128,1024] PSUM tiles. Softmax exp is
  split across engines: heads 0,1 exact exp on ACT; heads 2,3 Schraudolph
  fast-exp on DVE (tensor_scalar mult+add into an int16 view of the bf16 e
  tile; bf16-bit-trick softmax validated at ~8e-4 model rel err). AV
  col-packed 4-wide accumulates over kc into one PSUM bank; a col-packed
  all-ones round accumulates softmax denominators onto exactly the 32
  partitions of their head's AV rows, so normalization is one elementwise
  PSUM multiply by reciprocal_approx_fast of the denominators. The PE
  stream is software-pipelined one k-chunk ahead (scores(kc+1) issued
  before AV(kc)) so ACT/DVE never wait on the PE.
"""

import numpy as np

B, C1, C2, D, NH, H, W = 2, 64, 256, 256, 8, 64, 64
HD = D // NH                 # 32
N_TOK = H * W                # 4096 tokens per image
N_CORES = 8
CORES_PER_B = N_CORES // B   # 4
NQ = N_TOK // CORES_PER_B    # 1024 q tokens per core
EPS = 1e-5
SCALE = HD ** -0.5
KC = N_TOK // 128            # 32 k-chunks
QT_TILES = NQ // 128         # 8 q-tiles of 128
F1 = 4 * D                   # 1024 FFN hidden

# Schraudolph fast-exp in bf16 bits: bits_i16 = s*SCALE*(128/ln2) + C2
FEXP_C1 = SCALE * 128.0 / np.log(2.0)
FEXP_C2 = 16252.0

_built = None


def _build():
    from contextlib import ExitStack

    import concourse.mybir as mybir
    import concourse.tile as tile
    from concourse import bacc
    from concourse.masks import make_identity

    F32 = mybir.dt.float32
    F32R = mybir.dt.float32r
    BF16 = mybir.dt.bfloat16
    I16 = mybir.dt.int16
    AF = mybir.ActivationFunctionType
    OP = mybir.AluOpType

    nc = bacc.Bacc(trn_type="TRN2", target_bir_lowering=False, debug=False,
                   num_devices=N_CORES)

    # ---- DRAM I/O ----
    xq = nc.dram_tensor("xq", [C1, NQ], BF16, kind="ExternalInput").ap()
    xc = nc.dram_tensor("xc", [C2, N_TOK], BF16, kind="ExternalInput").ap()
    wkt = nc.dram_tensor("wkt", [C2, D], BF16, kind="ExternalInput").ap()
    wvt = nc.dram_tensor("wvt", [C2, D], BF16, kind="ExternalInput").ap()
    wqt = nc.dram_tensor("wqt", [C1, D], BF16, kind="ExternalInput").ap()
    wrt = nc.dram_tensor("wrt", [C1, D], BF16, kind="ExternalInput").ap()
    wot = nc.dram_tensor("wot", [D, D], BF16, kind="ExternalInput").ap()
    w1t = nc.dram_tensor("w1t", [D, F1], BF16, kind="ExternalInput").ap()
    w2t = nc.dram_tensor("w2t", [F1, D], BF16, kind="ExternalInput").ap()
    g1 = nc.dram_tensor("g1", [D], F32, kind="ExternalInput").ap()
    b1 = nc.dram_tensor("b1", [D], F32, kind="ExternalInput").ap()
    g2 = nc.dram_tensor("g2", [D], F32, kind="ExternalInput").ap()
    b2 = nc.dram_tensor("b2", [D], F32, kind="ExternalInput").ap()
    bf1 = nc.dram_tensor("bf1", [F1], F32, kind="ExternalInput").ap()
    bf2 = nc.dram_tensor("bf2", [D], F32, kind="ExternalInput").ap()
    out = nc.dram_tensor("out", [NQ, D], F32, kind="ExternalOutput").ap()

    with tile.TileContext(nc) as tc, ExitStack() as ctx:
        # ---- persistent SBUF ----
        P = ctx.enter_context(tc.tile_pool(name="persist", bufs=1))

        xq_sb = P.tile([C1, NQ], BF16, name="xq_sb")
        wot_sb = [P.tile([128, D], BF16, name=f"wot{c}") for c in range(2)]
        w1t_sb = [P.tile([128, F1], BF16, name=f"w1t{c}") for c in range(2)]
        w2t_sb = P.tile([128, 8, D], BF16, name="w2t_sb")
        bf1_col = P.tile([128, 8], F32, name="bf1_col")
        kt_sb = [P.tile([128, N_TOK], BF16, name=f"kt{g}") for g in range(2)]
        v_sb = P.tile([128, KC, D], BF16, name="v_sb")
        qt_sb = [P.tile([128, NQ], BF16, name=f"qt{g}") for g in range(2)]
        resid_sb = P.tile([128, QT_TILES, D], F32, name="resid_sb")
        attn_sb = [P.tile([128, NQ], BF16, name=f"attn{g}") for g in range(2)]
        ones_sb = P.tile([128, HD], BF16, name="ones_sb")
        ident = P.tile([128, 128], F32, name="ident")
        eps_sb = P.tile([128, 1], F32, name="eps_sb")
        g1_bc = P.tile([128, D], F32, name="g1_bc")
        b1_bc = P.tile([128, D], F32, name="b1_bc")
        g2_bc = P.tile([128, D], F32, name="g2_bc")
        b2_bc = P.tile([128, D], F32, name="b2_bc")
        bf2_bc = P.tile([128, D], F32, name="bf2_bc")
        x1_sb = P.tile([128, QT_TILES, D], F32, name="x1_sb")
        x1t_sb = [P.tile([128, NQ], BF16, name=f"x1t{g}") for g in range(2)]
        hdn_sb = P.tile([128, 8, NQ], BF16, name="hdn_sb")

        nc.vector.memset(ones_sb, 1.0)
        nc.vector.memset(eps_sb, EPS)
        make_identity(nc, ident)

        def bcast_row(eng, dst, src_ap):
            # (n,) dram -> (128, n) sbuf, replicated on all partitions
            import concourse.bass as bass
            src = bass.AP(tensor=src_ap.tensor, offset=src_ap.offset,
                          ap=[[0, 128]] + src_ap.ap)
            eng.dma_start(dst, src)

        # critical-path DMAs on the SP queue: q-side, then xc (chunked).
        nc.sync.dma_start(xq_sb, xq)

        # =============== Phase A: projections ===============
        NTH = N_TOK // 2  # token-half for chunked xc arrival
        with tc.tile_pool(name="xc_pool", bufs=1) as XP, \
             tc.tile_pool(name="psA", bufs=2, space="PSUM") as psA:
            wqt_sb = XP.tile([C1, D], BF16, name="wqt_sb")
            wrt_sb = XP.tile([C1, D], BF16, name="wrt_sb")
            wkt_sb = [XP.tile([128, D], BF16, name=f"wkt{c}")
                      for c in range(2)]
            wvt_sb = [XP.tile([128, D], BF16, name=f"wvt{c}")
                      for c in range(2)]
            xc_sb = [XP.tile([128, N_TOK], BF16, name=f"xc{c}")
                     for c in range(2)]
            nc.sync.dma_start(wqt_sb, wqt)
            nc.sync.dma_start(wrt_sb, wrt)
            for c in range(2):
                nc.sync.dma_start(wkt_sb[c], wkt[128 * c:128 * (c + 1), :])
                nc.sync.dma_start(wvt_sb[c], wvt[128 * c:128 * (c + 1), :])
            # token-chunked arrival: both channel halves of tokens [0,2048)
            # first, so K/V projections start at ~half the xc DMA time.
            for th in range(2):
                for c in range(2):
                    nc.sync.dma_start(
                        xc_sb[c][:, NTH * th:NTH * (th + 1)],
                        xc[128 * c:128 * (c + 1), NTH * th:NTH * (th + 1)])
            # late-needed weights on the gpsimd SWDGE queue (keeps ACT free)
            for c in range(2):
                nc.gpsimd.dma_start(wot_sb[c], wot[128 * c:128 * (c + 1), :])
                nc.gpsimd.dma_start(w1t_sb[c], w1t[128 * c:128 * (c + 1), :])
            nc.gpsimd.dma_start(
                w2t_sb, w2t.rearrange("(a p) d -> p a d", p=128))
            nc.gpsimd.dma_start(bf1_col, bf1.rearrange("(a p) -> p a", p=128))
            bcast_row(nc.gpsimd, g1_bc, g1)
            bcast_row(nc.gpsimd, b1_bc, b1)
            bcast_row(nc.gpsimd, g2_bc, g2)
            bcast_row(nc.gpsimd, b2_bc, b2)
            bcast_row(nc.gpsimd, bf2_bc, bf2)

            # PE warm-up: dummy f32 matmuls ramp the tensor engine's
            # p-state from cold (0.65GHz) to full while input DMAs land.
            wu = psA.tile([128, 512], F32, name="qp")
            for i in range(12):
                nc.tensor.matmul(wu[:, 0:128], ident, ident[:, 0:128],
                                 start=True, stop=True)
            # QT[d,q] = sum_c WqT[c,d] * xqT[c,q]  (only needs xq)
            for g in range(2):
                for qs in range(NQ // 512):
                    qp = psA.tile([128, 512], F32, name="qp")
                    nc.tensor.matmul(
                        qp, wqt_sb[:, 128 * g:128 * (g + 1)],
                        xq_sb[:, 512 * qs:512 * (qs + 1)],
                        start=True, stop=True)
                    nc.scalar.copy(
                        qt_sb[g][:, 512 * qs:512 * (qs + 1)], qp)
            # resid[q,d] = sum_c xqT[c,q] * WresT[c,d]
            for qt_i in range(QT_TILES):
                rp = psA.tile([128, D], F32, name="rp")
                nc.tensor.matmul(
                    rp, xq_sb[:, 128 * qt_i:128 * (qt_i + 1)],
                    wrt_sb, start=True, stop=True)
                nc.vector.tensor_copy(resid_sb[:, qt_i, :], rp)

            # KT[d,k] = sum_c WkT[c,d] * xcT[c,k];  V[k,d] bf16.
            # kp/vp interleaved and their PSUM->SBUF copies alternate between
            # ACT and DVE so both engine queues drain concurrently.
            for th in range(2):
                for ks in range(4):
                    for g in range(2):
                        kk = 4 * th + ks
                        kp = psA.tile([128, 512], F32, name="kp")
                        for c in range(2):
                            nc.tensor.matmul(
                                kp, wkt_sb[c][:, 128 * g:128 * (g + 1)],
                                xc_sb[c][:, 512 * kk:512 * (kk + 1)],
                                start=(c == 0), stop=(c == 1))
                        dst = kt_sb[g][:, 512 * kk:512 * (kk + 1)]
                        if g == 0:
                            nc.scalar.copy(dst, kp)
                        else:
                            nc.vector.tensor_copy(dst, kp)
                    for vi in range(4):
                        kt_i = 16 * th + 4 * ks + vi
                        vp = psA.tile([128, D], F32, name="vp")
                        for c in range(2):
                            nc.tensor.matmul(
                                vp, xc_sb[c][:, 128 * kt_i:128 * (kt_i + 1)],
                                wvt_sb[c], start=(c == 0), stop=(c == 1))
                        if vi % 2 == 0:
                            nc.vector.tensor_copy(v_sb[:, kt_i, :], vp)
                        else:
                            nc.scalar.copy(v_sb[:, kt_i, :], vp)

        # =============== Phase B: attention ===============
        with tc.tile_pool(name="scps", bufs=3, space="PSUM") as scps, \
             tc.tile_pool(name="avps", bufs=1, space="PSUM") as avps, \
             tc.tile_pool(name="epool", bufs=3) as epool, \
             tc.tile_pool(name="nrm", bufs=2) as nrm:
            for qc in range(2):
                qs = slice(512 * qc, 512 * (qc + 1))
                for g in range(2):
                    av = avps.tile([128, 512], F32, name="av")
                    ao = avps.tile([128, 512], F32, name="ao")

                    def scores(kc):
                        ks = slice(128 * kc, 128 * (kc + 1))
                        sc = [scps.tile([128, 1024], F32, name="sc")
                              for i in range(2)]
                        for h in range(4):
                            p = 32 * h
                            nc.tensor.matmul(
                                sc[h // 2][:, 512 * (h % 2):512 * (h % 2 + 1)],
                                kt_sb[g][p:p + 32, ks],
                                qt_sb[g][p:p + 32, qs],
                                start=True, stop=True, tile_position=(p, 0))
                        e = epool.tile([128, 4 * 512], BF16, name="e")
                        # heads 0,1: exact exp on ACT
                        nc.scalar.activation(
                            e[:, 0:1024], sc[0], AF.Exp, scale=SCALE)
                        # heads 2,3: Schraudolph fast-exp on DVE
                        nc.vector.tensor_scalar(
                            out=e[:, 1024:2048].bitcast(I16), in0=sc[1],
                            scalar1=float(FEXP_C1), scalar2=float(FEXP_C2),
                            op0=OP.mult, op1=OP.add)
                        return e

                    e_cur = scores(0)
                    for kc in range(KC):
                        e_next = scores(kc + 1) if kc + 1 < KC else None
                        st, sp = (kc == 0), (kc == KC - 1)
                        for h in range(4):
                            p = 32 * h
                            es = e_cur[:, 512 * h:512 * (h + 1)]
                            nc.tensor.matmul(
                                av[p:p + 32, :],
                                v_sb[:, kc, HD * (4 * g + h):HD * (4 * g + h) + HD],
                                es, start=st, stop=sp,
                                tile_position=(0, p), skip_group_check=True)
                        for h in range(4):
                            p = 32 * h
                            es = e_cur[:, 512 * h:512 * (h + 1)]
                            nc.tensor.matmul(
                                ao[p:p + 32, :], ones_sb, es,
                                start=st, stop=sp,
                                tile_position=(0, p), skip_group_check=True)
                        e_cur = e_next

                    rec = nrm.tile([128, 512], F32, name="rec")
                    nc.vector.reciprocal_approx_fast(out=rec, in_=ao)
                    nc.vector.tensor_mul(attn_sb[g][:, qs], av, rec)

        # =============== Phase C: Wo + LN1 + transpose ===============
        with tc.tile_pool(name="ptp", bufs=2, space="PSUM") as ptp, \
             tc.tile_pool(name="hpp", bufs=2, space="PSUM") as hpp, \
             tc.tile_pool(name="psD", bufs=1, space="PSUM") as psD, \
             tc.tile_pool(name="lnp", bufs=4) as lnp, \
             tc.tile_pool(name="lnagg", bufs=1) as lnagg:
            mv_all = lnagg.tile([128, QT_TILES, 2], F32, name="mv_all")
            rstd_all = lnagg.tile([128, QT_TILES], F32, name="rstd_all")
            xp_all = lnagg.tile([128, QT_TILES, D], F32, name="xp_all")
            for qt_i in range(QT_TILES):
                pp = ptp.tile([128, D], F32, name="pt")
                ts = slice(128 * qt_i, 128 * (qt_i + 1))
                for g in range(2):
                    nc.tensor.matmul(pp, attn_sb[g][:, ts], wot_sb[g],
                                     start=(g == 0), stop=(g == 1))
                xp = xp_all[:, qt_i, :]
                nc.vector.tensor_add(xp, pp, resid_sb[:, qt_i, :])
                stats = lnp.tile([128, 6], F32, name="stats")
                nc.vector.bn_stats(out=stats, in_=xp)
                nc.vector.bn_aggr(out=mv_all[:, qt_i, :], in_=stats)
            # batched rstd for all 8 tiles: one sqrt + one fast reciprocal
            sq = lnagg.tile([128, QT_TILES], F32, name="sq")
            nc.scalar.activation(sq, mv_all[:, :, 1], AF.Sqrt, bias=eps_sb)
            nc.vector.reciprocal_approx_fast(out=rstd_all, in_=sq)
            for qt_i in range(QT_TILES):
                ts = slice(128 * qt_i, 128 * (qt_i + 1))
                x1s = x1_sb[:, qt_i, :]
                nc.vector.tensor_scalar(
                    out=x1s, in0=xp_all[:, qt_i, :],
                    scalar1=mv_all[:, qt_i, 0:1],
                    scalar2=rstd_all[:, qt_i:qt_i + 1],
                    op0=OP.subtract, op1=OP.mult)
                nc.vector.tensor_mul(x1s, x1s, g1_bc)
                nc.vector.tensor_add(x1s, x1s, b1_bc)
                for dc in range(2):
                    tp = ptp.tile([128, D], F32, name="pt")[:, 0:128]
                    nc.tensor.transpose(
                        tp, x1_sb[:, qt_i, 128 * dc:128 * (dc + 1)], ident)
                    nc.scalar.copy(x1t_sb[dc][:, ts], tp)

            # =============== Phase D: FFN + LN2 ===============
            # hdn^T[f,q] = relu(sum_d W1T[d,f] x1T[d,q] + bf1[f]), relu
            # split ACT/DVE; FFN2 accumulation for tiles 0-3 rides one fc
            # step behind FFN1 in the same PE stream.
            fp_half = [psD.tile([128, D], F32, name=f"fph{i}")
                       for i in range(4)]
            for fc in range(8):
                for qcb in range(NQ // 512):
                    qsl = slice(512 * qcb, 512 * (qcb + 1))
                    hp_ = hpp.tile([128, 512], F32, name="hp_")
                    for dc in range(2):
                        nc.tensor.matmul(
                            hp_, w1t_sb[dc][:, 128 * fc:128 * (fc + 1)],
                            x1t_sb[dc][:, qsl], start=(dc == 0), stop=(dc == 1))
                    if fc % 2 == 0:
                        nc.scalar.activation(
                            hdn_sb[:, fc, qsl], hp_, AF.Relu,
                            bias=bf1_col[:, fc:fc + 1])
                    else:
                        nc.vector.tensor_scalar(
                            out=hdn_sb[:, fc, qsl], in0=hp_,
                            scalar1=bf1_col[:, fc:fc + 1], scalar2=0.0,
                            op0=OP.add, op1=OP.max)
                if fc >= 2:
                    for qt_i in range(4):
                        ts = slice(128 * qt_i, 128 * (qt_i + 1))
                        nc.tensor.matmul(
                            fp_half[qt_i], hdn_sb[:, fc - 2, ts],
                            w2t_sb[:, fc - 2, :],
                            start=(fc == 2), stop=False,
                            skip_group_check=True)
            for fc in range(6, 8):
                for qt_i in range(4):
                    ts = slice(128 * qt_i, 128 * (qt_i + 1))
                    nc.tensor.matmul(
                        fp_half[qt_i], hdn_sb[:, fc, ts], w2t_sb[:, fc, :],
                        start=False, stop=(fc == 7),
                        skip_group_check=True)
            # ffn[q,d] = sum_f hdnT[f,q] W2T[f,d]; x2 = LN2(x1+ffn+bf2)
            mv2_all = lnagg.tile([128, QT_TILES, 2], F32, name="mv2_all")
            rstd2_all = lnagg.tile([128, QT_TILES], F32, name="rstd2_all")
            xp2_all = lnagg.tile([128, QT_TILES, D], F32, name="xp2_all")
            for qt_i in range(QT_TILES):
                ts = slice(128 * qt_i, 128 * (qt_i + 1))
                if qt_i < 4:
                    fp = fp_half[qt_i]
                else:
                    if qt_i == 4:
                        fp_half = [psD.tile([128, D], F32, name=f"fph{i}")
                                   for i in range(4)]
                    fp = fp_half[qt_i - 4]
                    for fc in range(8):
                        nc.tensor.matmul(
                            fp, hdn_sb[:, fc, ts], w2t_sb[:, fc, :],
                            start=(fc == 0), stop=(fc == 7),
                            skip_group_check=True)
                xp2 = xp2_all[:, qt_i, :]
                nc.vector.tensor_add(xp2, fp, x1_sb[:, qt_i, :])
                nc.vector.tensor_add(xp2, xp2, bf2_bc)
                stats2 = lnp.tile([128, 6], F32, name="stats2")
                nc.vector.bn_stats(out=stats2, in_=xp2)
                nc.vector.bn_aggr(out=mv2_all[:, qt_i, :], in_=stats2)
            sq2 = lnagg.tile([128, QT_TILES], F32, name="sq2")
            nc.scalar.activation(sq2, mv2_all[:, :, 1], AF.Sqrt, bias=eps_sb)
            nc.vector.reciprocal_approx_fast(out=rstd2_all, in_=sq2)
            for qt_i in range(QT_TILES):
                ts = slice(128 * qt_i, 128 * (qt_i + 1))
                xo = lnp.tile([128, D], F32, name="xo")
                nc.vector.tensor_scalar(
                    out=xo, in0=xp2_all[:, qt_i, :],
                    scalar1=mv2_all[:, qt_i, 0:1],
                    scalar2=rstd2_all[:, qt_i:qt_i + 1],
                    op0=OP.subtract, op1=OP.mult)
                nc.vector.tensor_mul(xo, xo, g2_bc)
                nc.vector.tensor_add(xo, xo, b2_bc)
                nc.sync.dma_start(out[ts, :], xo)

    nc.compile()
    return nc


def _get_nc():
    global _built
    if _built is None:
        _built = _build()
    return _built


def _make_in_maps(inputs):
    f32 = np.float32
    F_lidar = np.ascontiguousarray(inputs["F_lidar"], dtype=f32)
    F_cam = np.ascontiguousarray(inputs["F_cam"], dtype=f32)
    import ml_dtypes
    bf16 = ml_dtypes.bfloat16
    common = {
        "wkt": np.ascontiguousarray(inputs["Wk"].T).astype(bf16),
        "wvt": np.ascontiguousarray(inputs["Wv"].T).astype(bf16),
        "wqt": np.ascontiguousarray(inputs["Wq"].T).astype(bf16),
        "wrt": np.ascontiguousarray(inputs["Wres"].T).astype(bf16),
        "wot": np.ascontiguousarray(inputs["Wo"].T).astype(bf16),
        "w1t": np.ascontiguousarray(inputs["W1"].T).astype(bf16),
        "w2t": np.ascontiguousarray(inputs["W2"].T).astype(bf16),
        "g1": np.asarray(inputs["g1"], f32), "b1": np.asarray(inputs["b1"], f32),
        "g2": np.asarray(inputs["g2"], f32), "b2": np.asarray(inputs["b2"], f32),
        "bf1": np.asarray(inputs["bf1"], f32),
        "bf2": np.asarray(inputs["bf2"], f32),
    }
    in_maps = []
    for c in range(N_CORES):
        b, s = c // CORES_PER_B, (c % CORES_PER_B) * NQ
        m = dict(common)
        m["xq"] = np.ascontiguousarray(
            F_lidar[b].reshape(C1, N_TOK)[:, s:s + NQ]).astype(bf16)
        m["xc"] = np.ascontiguousarray(
            F_cam[b].reshape(C2, N_TOK)).astype(bf16)
        in_maps.append(m)
    return in_maps


def kernel(**inputs):
    from concourse.bass_utils import run_bass_kernel_spmd

    nc = _get_nc()
    in_maps = _make_in_maps(inputs)
    res = run_bass_kernel_spmd(nc, in_maps, list(range(N_CORES)))
    out = np.empty((B, D, N_TOK), dtype=np.float32)
    for c in range(N_CORES):
        b, s = c // CORES_PER_B, (c % CORES_PER_B) * NQ
        out[b, :, s:s + NQ] = res.results[c]["out"].T
    return out.reshape(B, D, H, W)
